# revision 26
# baseline (speedup 1.0000x reference)
"""Trainium2 Bass kernel for a 2-layer spiking NN (snntorch Leaky, reset='subtract').

Reference semantics (per time step t, fp32):
    cur1 = x_t @ w1.T + b1
    mem1 = beta*mem1 + cur1 - spk1          ; spk1 = (mem1 - 1 > 0)
    cur2 = spk1 @ w2.T + b2
    mem2 = beta*mem2 + cur2 - spk2          ; spk2 = (mem2 - 1 > 0)
    out  = sum_t spk2                        # [B, OUT] spike counts

Shapes: T=25, B=1024, IN=1024, HID=4096, OUT=64.  Data-parallel over 8
NeuronCores: each core runs 128 batch rows; outputs are concatenated.

Layer-1 runs in a transposed layout: matmul output is [hid-neuron, batch],
with 2-3 timesteps batched into the moving operand (x is input data, so all
cur1 are state-independent).  Weights split as w = fp16(w) + loA/S + loB/S
with loA/loB fp8-e4m3 residuals (S=2^16) fed through DoubleRow matmuls whose
moving operand is the binary x encoded as {0, 2^-16} in fp8-e5m2 (exact
subnormal), so lo products accumulate directly onto the fp16 PSUM group at
0.5 cycles/row.  Total PE cost 1.5 cyc/row vs 2.0 for double-fp16.

Membrane recurrence (bias folded: m = mem - c, c = b/(1-beta), th = 1-c,
state s = beta*m - spk, step: q = s + cur ; spk = (q > th) ; s = beta*q - spk)
is pipelined across four engines per 4-chunk wave:
    DVE:  s1 += cur (strided PSUM read)     ACT:  sigma = Sign(s1 - th)
    Pool: spk = max(sigma, 0)               DVE:  s1 = beta*s1 - spk
Layer-2 (small) keeps the batch-on-partitions layout: stationary = spk chunks
(already transposed "for free"), moving = packed fp16 [w2hi || w2lo] slices.
"""

import os
import sys

for _p in ("/opt/trn_rl_repo", "/opt/pypackages"):
    if os.path.isdir(_p) and _p not in sys.path:
        sys.path.insert(0, _p)

import numpy as np
import ml_dtypes

import concourse.bacc as bacc
import concourse.mybir as mybir
import concourse.tile as tile
from concourse.bass_utils import run_bass_kernel_spmd

T, B, IN, HID, OUT = 25, 1024, 1024, 4096, 64
NCORES = 8
BC = B // NCORES          # 128 batch rows per core
KC = IN // 128            # 8 contraction chunks (hi pass)
JC = IN // 256            # 4 contraction chunk-pairs (DoubleRow lo pass)
NCH = HID // 128          # 32 neuron chunks
L2J = HID // 128          # 32 layer-2 contraction chunks
BETA = 0.9
S1 = float(2 ** 16)       # layer-1 lo-residual scale (paired with x8 = 2^-16)
N_LO = int(os.environ.get("KERNEL_NLO", "1"))  # e4m3 residual terms for w1
S2 = float(2 ** 12)       # layer-2 fp16 lo-split scale (baseline scheme)
X8V = np.float32(2.0 ** -16)

F32 = mybir.dt.float32
F16 = mybir.dt.float16
F8E4 = mybir.dt.float8e4
F8E5 = mybir.dt.float8e5
DRMODE = mybir.MatmulPerfMode.DoubleRow
E4NP = ml_dtypes.float8_e4m3
E5NP = ml_dtypes.float8_e5m2
ADD = mybir.AluOpType.add
MULT = mybir.AluOpType.mult
SUB = mybir.AluOpType.subtract
ISGT = mybir.AluOpType.is_gt
MAX = mybir.AluOpType.max
SIGN = mybir.ActivationFunctionType.Sign

# time groups: one triple first (absorbs the weight-DMA ramp at a lower
# PE consumption rate), then 11 pairs
GROUPS = [(0, 3)] + [(3 + 2 * i, 2) for i in range(11)]
XCOLS = 8 * 128 * T       # flat x dram cols (k-major within each group)

_CACHE: dict = {}


def _build_nc():
    nc = bacc.Bacc("TRN2", target_bir_lowering=False, debug=False)

    w1hi_d = nc.dram_tensor("w1hi", [128, NCH * 1024], F16, kind="ExternalInput")
    w1la_d = nc.dram_tensor("w1la", [128, NCH * 1024], F8E4, kind="ExternalInput")
    w1lb_d = (nc.dram_tensor("w1lb", [128, NCH * 1024], F8E4, kind="ExternalInput")
              if N_LO == 2 else None)
    x16_d = nc.dram_tensor("x16", [128, XCOLS], F16, kind="ExternalInput")
    x8_d = nc.dram_tensor("x8", [128, XCOLS], F8E5, kind="ExternalInput")
    # smalls packed: [nth | s10 | th2 | s20] = 32+32+64+64 f32 cols
    sml_d = nc.dram_tensor("sml", [128, 2 * NCH + 2 * OUT], F32, kind="ExternalInput")
    w2c_d = nc.dram_tensor("w2c", [128, L2J * 128], F16, kind="ExternalInput")
    out_d = nc.dram_tensor("out", [128, OUT], F32, kind="ExternalOutput")

    with tile.TileContext(nc) as tc:
        with (
            tc.tile_pool(name="const", bufs=1) as cpool,
            tc.tile_pool(name="state", bufs=1) as spool,
            tc.tile_pool(name="xin", bufs=2) as xpool,
            tc.tile_pool(name="sg", bufs=4) as gpool,
            tc.tile_pool(name="spk", bufs=9) as kpool,
            tc.tile_pool(name="small", bufs=2) as mpool,
            tc.tile_pool(name="l1p", bufs=2, space="PSUM") as l1pool,
            tc.tile_pool(name="po2", bufs=4, space="PSUM") as o2pool,
        ):
            w1hi = cpool.tile([128, NCH, KC, 128], F16, tag="w1hi")
            w1la = cpool.tile([128, NCH, JC, 2, 128], F8E4, tag="w1la")
            w1lb = (cpool.tile([128, NCH, JC, 2, 128], F8E4, tag="w1lb")
                    if N_LO == 2 else None)
            nth = cpool.tile([128, NCH], F32, tag="nth")
            s10 = cpool.tile([128, NCH], F32, tag="s10")
            w2c = cpool.tile([128, L2J * 128], F16, tag="w2c")
            th2 = cpool.tile([128, OUT], F32, tag="th2")
            s1 = spool.tile([128, NCH, 128], F32, tag="s1")
            s2 = spool.tile([128, OUT], F32, tag="s2")
            cnt = spool.tile([128, OUT], F32, tag="cnt")

            # ---- init DMAs (one packed transfer for the small constants) ----
            sml = cpool.tile([128, 2 * NCH + 2 * OUT], F32, tag="sml")
            nc.sync.dma_start(sml[:], sml_d[:, :])
            nc.vector.tensor_copy(nth[:], sml[:, :NCH])
            nc.vector.tensor_copy(s10[:], sml[:, NCH:2 * NCH])
            nc.vector.tensor_copy(th2[:], sml[:, 2 * NCH:2 * NCH + OUT])
            nc.vector.tensor_copy(s2[:], sml[:, 2 * NCH + OUT:])
            # weights stream in 16 per-2-chunk pieces, emitted just-in-time in
            # the wave loop (Pool queue: hi; SP queue: lo terms) so no engine
            # sequencer is blocked by a burst of DMA issues at startup
            def load_w_piece(q):
                cs = slice(q * 2048, (q + 1) * 2048)
                nc.gpsimd.dma_start(w1hi[:, q * 2:(q + 1) * 2, :, :], w1hi_d[:, cs])
                nc.sync.dma_start(w1la[:, q * 2:(q + 1) * 2, :, :, :], w1la_d[:, cs])
                if N_LO == 2:
                    nc.sync.dma_start(w1lb[:, q * 2:(q + 1) * 2, :, :, :], w1lb_d[:, cs])

            load_w_piece(0)
            # s1 state init: s1[p, c, b] = s10[p, c]
            nc.vector.memset(s1[:], 0.0)
            nc.vector.memset(cnt[:], 0.0)
            for c in range(NCH):
                nc.vector.tensor_scalar(s1[:, c, :], s1[:, c, :], s10[:, c:c + 1], None, ADD)

            # ---- wave descriptors (flat across groups) ----
            waves = []
            for gi, (t0, glen) in enumerate(GROUPS):
                cpw = 4 if glen == 2 else 2
                nw = NCH // cpw
                for w in range(nw):
                    waves.append(dict(
                        g=gi, glen=glen, cpw=cpw, c0=w * cpw,
                        first=(w == 0), last=(w == nw - 1)))
            NW = len(waves)

            xt16 = [None] * len(GROUPS)
            xt8 = [None] * len(GROUPS)
            xoff = [0] * len(GROUPS)
            off = 0
            for gi, (t0, glen) in enumerate(GROUPS):
                xoff[gi] = off
                off += 8 * glen * 128

            def load_x(gi):
                glen = GROUPS[gi][1]
                xt = xpool.tile([128, KC, glen * 128], F16, tag="x16", name=f"x16_{gi}")
                x8t = xpool.tile([128, JC, 2, glen * 128], F8E5, tag="x8", name=f"x8_{gi}")
                o = xoff[gi]
                nc.sync.dma_start(xt[:], x16_d[:, o:o + 8 * glen * 128])
                nc.gpsimd.dma_start(x8t[:], x8_d[:, o:o + 8 * glen * 128])
                xt16[gi], xt8[gi] = xt, x8t

            load_x(0)

            # per-wave tile handles, filled during emission
            Pt = [None] * NW                      # l1 psum tile
            SG = [[None] * 3 for _ in range(NW)]  # sigma tiles per tl
            SPK = [[None] * 3 for _ in range(NW)]  # spike tiles per tl
            o2t = {}                              # (gi, tl) -> l2 psum tile

            def emit_l1(i):
                wv = waves[i]
                gi, cpw, c0, glen = wv["g"], wv["cpw"], wv["c0"], wv["glen"]
                if wv["first"] and gi >= 1 and gi + 1 < len(GROUPS):
                    load_x(gi + 1)
                shape = [128, cpw, glen, 128]
                pad = [128, cpw, glen, 128] if glen == 2 else [128, cpw, 4, 128]
                P = l1pool.tile(shape, F32, tag="P", name=f"P_{i}", padded_shape=pad)
                Pt[i] = P
                xt, x8t = xt16[gi], xt8[gi]
                for ci in range(cpw):
                    c = c0 + ci
                    for k in range(KC):
                        nc.tensor.matmul(P[:, ci, :, :], w1hi[:, c, k, :], xt[:, k, :],
                                         start=(k == 0), stop=False)
                    los = (w1la, w1lb) if N_LO == 2 else (w1la,)
                    for lo in los:
                        for j in range(JC):
                            nc.tensor.matmul(P[:, ci, :, :], lo[:, c, j, :, :],
                                             x8t[:, j, :, :], start=False,
                                             stop=(lo is los[-1] and j == JC - 1),
                                             perf_mode=DRMODE)

            # All recurrence ops are per-chunk: each chunk's op1 -> sigma ->
            # spk -> op3 chain completes shortly after that chunk's matmuls,
            # so the psum tile recycles ~2us after the wave instead of riding
            # a 4-chunk grouped barrier through three engines.
            def emit_op1(j, tl):
                wv = waves[j]
                c0, cpw = wv["c0"], wv["cpw"]
                for ci in range(cpw):
                    sl = s1[:, c0 + ci, :]
                    nc.vector.tensor_tensor(sl, sl, Pt[j][:, ci, tl, :], ADD)

            def emit_op3(j, tl):
                wv = waves[j]
                c0, cpw = wv["c0"], wv["cpw"]
                for ci in range(cpw):
                    sl = s1[:, c0 + ci, :]
                    nc.vector.scalar_tensor_tensor(sl, sl, BETA, SPK[j][tl][:, ci, :],
                                                   MULT, SUB)

            def emit_sigma(j, tl):
                wv = waves[j]
                c0, cpw = wv["c0"], wv["cpw"]
                sg = gpool.tile([128, cpw, 128], F16, tag="sg", name=f"sg_{j}_{tl}")
                SG[j][tl] = sg
                for ci in range(cpw):
                    c = c0 + ci
                    nc.scalar.activation(sg[:, ci, :], s1[:, c, :], SIGN,
                                         bias=nth[:, c:c + 1], scale=1.0)

            def emit_spk(j, tl):
                wv = waves[j]
                cpw = wv["cpw"]
                sp = kpool.tile([128, cpw, 128], F16, tag="spk", name=f"spk_{j}_{tl}")
                SPK[j][tl] = sp
                for ci in range(cpw):
                    nc.gpsimd.tensor_scalar(sp[:, ci, :], SG[j][tl][:, ci, :],
                                            0.0, None, MAX)

            def emit_l2(j, tl):
                wv = waves[j]
                gi, c0, cpw = wv["g"], wv["c0"], wv["cpw"]
                if c0 == 0:
                    o2t[(gi, tl)] = o2pool.tile([128, 128], F32, tag="o2",
                                                name=f"o2_{gi}_{tl}")
                o2 = o2t[(gi, tl)]
                sp = SPK[j][tl]
                for ci in range(cpw):
                    jj = c0 + ci
                    nc.tensor.matmul(o2[:], sp[:, ci, :], w2c[:, jj * 128:(jj + 1) * 128],
                                     start=(jj == 0), stop=(jj == L2J - 1))

            def emit_l2rec(gi, tl):
                # returns a list of closures so the caller can spread the DVE
                # work over several slots (keeps DVE under its slot budget)
                o2 = o2t.pop((gi, tl))
                q2 = mpool.tile([128, OUT], F32, tag="q2", name=f"q2_{gi}_{tl}")
                spk2 = mpool.tile([128, OUT], F32, tag="spk2", name=f"spk2_{gi}_{tl}")
                return [
                    lambda: nc.vector.scalar_tensor_tensor(q2[:], o2[:, OUT:], 1.0 / S2, s2[:], MULT, ADD),
                    lambda: nc.vector.tensor_tensor(q2[:], q2[:], o2[:, :OUT], ADD),
                    lambda: nc.vector.tensor_tensor(spk2[:], q2[:], th2[:], ISGT),
                    lambda: nc.vector.scalar_tensor_tensor(s2[:], q2[:], BETA, spk2[:], MULT, SUB),
                    lambda: nc.vector.tensor_tensor(cnt[:], cnt[:], spk2[:], ADD),
                ]

            def has_tl(j, d):
                return 0 <= j < NW and d < waves[j]["glen"]

            # ---- software-pipelined emission ----
            l2_emitted = set()
            l2rec_ops = []
            for i in range(NW + 8):
                # deferred layer-2 recurrence ops, at most 3 per slot so the
                # DVE ladder never exceeds its slot budget
                budget = 3 if i < NW else 16
                while l2rec_ops and budget > 0:
                    l2rec_ops.pop(0)()
                    budget -= 1
                if 0 <= i <= 14:
                    load_w_piece(i + 1)
                if i == 0:
                    nc.sync.dma_start(w2c[:], w2c_d[:, :])
                if i == 8:
                    load_x(1)
                if i < NW:
                    emit_l1(i)
                # DVE ladder: op3 one slot after spike, op1 chained after op3
                for d in range(1, 4):
                    j = i - d
                    if has_tl(j, d - 1):
                        emit_op3(j, d - 1)
                    if has_tl(j, d):
                        emit_op1(j, d)
                if i < NW:
                    emit_op1(i, 0)
                # ACT sigmas for the op1s emitted this slot (t0 last)
                for d in range(1, 4):
                    if has_tl(i - d, d):
                        emit_sigma(i - d, d)
                if i < NW:
                    emit_sigma(i, 0)
                # Pool spikes in same order
                for d in range(1, 4):
                    if has_tl(i - d, d):
                        emit_spk(i - d, d)
                if i < NW:
                    emit_spk(i, 0)
                # layer-2 matmuls: lag 2 slots behind spike production (lag 1
                # for the final waves so the tail drains sooner); then the
                # layer-2 recurrence immediately after a group's last chunk
                for j in range(max(0, i - 6), i + 1):
                    if not (0 <= j < NW):
                        continue
                    lag = 1 if j >= NW - 4 else 2
                    for tl in range(waves[j]["glen"]):
                        if (j, tl) in l2_emitted or i < j + tl + lag:
                            continue
                        l2_emitted.add((j, tl))
                        emit_l2(j, tl)
                        if waves[j]["last"]:
                            l2rec_ops.extend(emit_l2rec(waves[j]["g"], tl))

            nc.sync.dma_start(out_d[:, :], cnt[:])

    nc.compile()
    return nc


def _prep_shared(w1, b1, w2, b2):
    w1 = w1.astype(np.float32)
    hi16 = w1.astype(np.float16)
    hiF = hi16.astype(np.float32)
    resA = ((w1 - hiF) * np.float32(S1)).astype(E4NP)
    resB = ((w1 - hiF - resA.astype(np.float32) / np.float32(S1)) * np.float32(S1)).astype(E4NP)

    # w1hi: [p, c*1024 + k*128 + n] = hi16[c*128+n, k*128+p]
    def lay_hi(a):
        return np.ascontiguousarray(
            a.reshape(NCH, 128, KC, 128).transpose(3, 0, 2, 1).reshape(128, NCH * 1024))

    # lo: [p, c*1024 + j*256 + s*128 + n] = lo[c*128+n, j*256+s*128+p]
    def lay_lo(a):
        return np.ascontiguousarray(
            a.reshape(NCH, 128, JC, 2, 128).transpose(4, 0, 2, 3, 1).reshape(128, NCH * 1024))

    w2t = np.ascontiguousarray(w2.T.astype(np.float32))       # [HID, OUT]
    w2hi = w2t.astype(np.float16)
    w2lo = ((w2t - w2hi.astype(np.float32)) * np.float32(S2)).astype(np.float16)
    w2cat = np.concatenate([w2hi.reshape(L2J, 128, OUT), w2lo.reshape(L2J, 128, OUT)],
                           axis=2)
    w2c = np.ascontiguousarray(w2cat.transpose(1, 0, 2).reshape(128, L2J * 128))

    c1 = (b1.astype(np.float32) / np.float32(1.0 - BETA)).astype(np.float32)
    c2 = (b2.astype(np.float32) / np.float32(1.0 - BETA)).astype(np.float32)
    nth = np.ascontiguousarray((c1 - np.float32(1.0)).reshape(NCH, 128).T)  # -(1-c1)
    s10 = np.ascontiguousarray((-np.float32(BETA) * c1).reshape(NCH, 128).T)
    th2 = np.broadcast_to((1.0 - c2).astype(np.float32), (128, OUT))
    s20 = np.broadcast_to((-BETA * c2).astype(np.float32), (128, OUT))
    sml = np.ascontiguousarray(
        np.concatenate([nth, s10, th2, s20], axis=1).astype(np.float32))
    out = {"w1hi": lay_hi(hi16), "w1la": lay_lo(resA), "w2c": w2c, "sml": sml}
    if N_LO == 2:
        out["w1lb"] = lay_lo(resB)
    return out


def _prep_x(spike_seq, core):
    xs = np.asarray(spike_seq[:, core * BC:(core + 1) * BC, :], dtype=np.float32)
    x16_blocks, x8_blocks = [], []
    for t0, glen in GROUPS:
        blk = xs[t0:t0 + glen]                       # [glen, 128b, 1024in]
        b16 = blk.reshape(glen, 128, KC, 128).transpose(3, 2, 0, 1)   # [p, k, tl, b]
        x16_blocks.append(b16.reshape(128, KC * glen * 128))
        b8 = blk.reshape(glen, 128, JC, 2, 128).transpose(4, 2, 3, 0, 1)  # [p,j,s,tl,b]
        x8_blocks.append((b8 * X8V).reshape(128, JC * 2 * glen * 128))
    x16 = np.ascontiguousarray(np.concatenate(x16_blocks, axis=1)).astype(np.float16)
    x8 = np.ascontiguousarray(np.concatenate(x8_blocks, axis=1)).astype(E5NP)
    return x16, x8


def kernel(spike_seq, w1, b1, w2, b2):
    if "nc" not in _CACHE:
        _CACHE["nc"] = _build_nc()
    nc = _CACHE["nc"]

    shared = _prep_shared(np.asarray(w1), np.asarray(b1), np.asarray(w2), np.asarray(b2))
    in_maps = []
    for c in range(NCORES):
        x16, x8 = _prep_x(spike_seq, c)
        in_maps.append({"x16": x16, "x8": x8, **shared})
    res = run_bass_kernel_spmd(nc, in_maps, core_ids=list(range(NCORES)))
    out = np.concatenate([res.results[c]["out"] for c in range(NCORES)], axis=0)
    return out.astype(np.asarray(spike_seq).dtype)


# revision 28
# speedup vs baseline: 1.0158x; 1.0158x over previous
"""Trainium2 Bass kernel for a 2-layer spiking NN (snntorch Leaky, reset='subtract').

Reference semantics (per time step t, fp32):
    cur1 = x_t @ w1.T + b1
    mem1 = beta*mem1 + cur1 - spk1          ; spk1 = (mem1 - 1 > 0)
    cur2 = spk1 @ w2.T + b2
    mem2 = beta*mem2 + cur2 - spk2          ; spk2 = (mem2 - 1 > 0)
    out  = sum_t spk2                        # [B, OUT] spike counts

Shapes: T=25, B=1024, IN=1024, HID=4096, OUT=64.  Data-parallel over 8
NeuronCores: each core runs 128 batch rows; outputs are concatenated.

Layer-1 runs in a transposed layout: matmul output is [hid-neuron, batch],
with 2-3 timesteps batched into the moving operand (x is input data, so all
cur1 are state-independent).  Weights split as w = fp16(w) + loA/S + loB/S
with loA/loB fp8-e4m3 residuals (S=2^16) fed through DoubleRow matmuls whose
moving operand is the binary x encoded as {0, 2^-16} in fp8-e5m2 (exact
subnormal), so lo products accumulate directly onto the fp16 PSUM group at
0.5 cycles/row.  Total PE cost 1.5 cyc/row vs 2.0 for double-fp16.

Membrane recurrence (bias folded: m = mem - c, c = b/(1-beta), th = 1-c,
state s = beta*m - spk, step: q = s + cur ; spk = (q > th) ; s = beta*q - spk)
is pipelined across four engines per 4-chunk wave:
    DVE:  s1 += cur (strided PSUM read)     ACT:  sigma = Sign(s1 - th)
    Pool: spk = max(sigma, 0)               DVE:  s1 = beta*s1 - spk
Layer-2 (small) keeps the batch-on-partitions layout: stationary = spk chunks
(already transposed "for free"), moving = packed fp16 [w2hi || w2lo] slices.
"""

import os
import sys

for _p in ("/opt/trn_rl_repo", "/opt/pypackages"):
    if os.path.isdir(_p) and _p not in sys.path:
        sys.path.insert(0, _p)

import numpy as np
import ml_dtypes

import concourse.bacc as bacc
import concourse.mybir as mybir
import concourse.tile as tile
from concourse.bass_utils import run_bass_kernel_spmd

T, B, IN, HID, OUT = 25, 1024, 1024, 4096, 64
NCORES = 8
BC = B // NCORES          # 128 batch rows per core
KC = IN // 128            # 8 contraction chunks (hi pass)
JC = IN // 256            # 4 contraction chunk-pairs (DoubleRow lo pass)
NCH = HID // 128          # 32 neuron chunks
L2J = HID // 128          # 32 layer-2 contraction chunks
BETA = 0.9
S1 = float(2 ** 16)       # layer-1 lo-residual scale (paired with x8 = 2^-16)
N_LO = int(os.environ.get("KERNEL_NLO", "1"))  # e4m3 residual terms for w1
S2 = float(2 ** 12)       # layer-2 fp16 lo-split scale (baseline scheme)
X8V = np.float32(2.0 ** -16)

F32 = mybir.dt.float32
F16 = mybir.dt.float16
F8E4 = mybir.dt.float8e4
F8E5 = mybir.dt.float8e5
DRMODE = mybir.MatmulPerfMode.DoubleRow
E4NP = ml_dtypes.float8_e4m3
E5NP = ml_dtypes.float8_e5m2
ADD = mybir.AluOpType.add
MULT = mybir.AluOpType.mult
SUB = mybir.AluOpType.subtract
ISGT = mybir.AluOpType.is_gt
MAX = mybir.AluOpType.max
SIGN = mybir.ActivationFunctionType.Sign

# time groups: one triple first (absorbs the weight-DMA ramp at a lower
# PE consumption rate), then 11 pairs
GROUPS = [(0, 3)] + [(3 + 2 * i, 2) for i in range(11)]
XCOLS = 8 * 128 * T       # flat x dram cols (k-major within each group)

_CACHE: dict = {}


def _build_nc():
    nc = bacc.Bacc("TRN2", target_bir_lowering=False, debug=False)

    w1hi_d = nc.dram_tensor("w1hi", [128, NCH * 1024], F16, kind="ExternalInput")
    w1la_d = nc.dram_tensor("w1la", [128, NCH * 1024], F8E4, kind="ExternalInput")
    w1lb_d = (nc.dram_tensor("w1lb", [128, NCH * 1024], F8E4, kind="ExternalInput")
              if N_LO == 2 else None)
    x16_d = nc.dram_tensor("x16", [128, XCOLS], F16, kind="ExternalInput")
    x8_d = nc.dram_tensor("x8", [128, XCOLS], F8E5, kind="ExternalInput")
    # smalls packed: [nth | s10 | th2 | s20] = 32+32+64+64 f32 cols
    sml_d = nc.dram_tensor("sml", [128, 2 * NCH + 2 * OUT], F32, kind="ExternalInput")
    w2c_d = nc.dram_tensor("w2c", [128, L2J * 128], F16, kind="ExternalInput")
    out_d = nc.dram_tensor("out", [128, OUT], F32, kind="ExternalOutput")

    with tile.TileContext(nc) as tc:
        with (
            tc.tile_pool(name="const", bufs=1) as cpool,
            tc.tile_pool(name="state", bufs=1) as spool,
            tc.tile_pool(name="xin", bufs=2) as xpool,
            tc.tile_pool(name="sg", bufs=4) as gpool,
            tc.tile_pool(name="spk", bufs=9) as kpool,
            tc.tile_pool(name="small", bufs=2) as mpool,
            tc.tile_pool(name="l1p", bufs=2, space="PSUM") as l1pool,
            tc.tile_pool(name="po2", bufs=4, space="PSUM") as o2pool,
        ):
            w1hi = cpool.tile([128, NCH, KC, 128], F16, tag="w1hi")
            w1la = cpool.tile([128, NCH, JC, 2, 128], F8E4, tag="w1la")
            w1lb = (cpool.tile([128, NCH, JC, 2, 128], F8E4, tag="w1lb")
                    if N_LO == 2 else None)
            nth = cpool.tile([128, NCH], F32, tag="nth")
            s10 = cpool.tile([128, NCH], F32, tag="s10")
            w2c = cpool.tile([128, L2J * 128], F16, tag="w2c")
            th2 = cpool.tile([128, OUT], F32, tag="th2")
            s1 = spool.tile([128, NCH, 128], F32, tag="s1")
            s2 = spool.tile([128, OUT], F32, tag="s2")
            cnt = spool.tile([128, OUT], F32, tag="cnt")

            # ---- init DMAs (one packed transfer for the small constants) ----
            sml = cpool.tile([128, 2 * NCH + 2 * OUT], F32, tag="sml")
            nc.sync.dma_start(sml[:], sml_d[:, :])
            nc.vector.tensor_copy(nth[:], sml[:, :NCH])
            nc.vector.tensor_copy(s10[:], sml[:, NCH:2 * NCH])
            nc.vector.tensor_copy(th2[:], sml[:, 2 * NCH:2 * NCH + OUT])
            nc.vector.tensor_copy(s2[:], sml[:, 2 * NCH + OUT:])
            # weights stream in 16 per-2-chunk pieces, emitted just-in-time in
            # the wave loop (Pool queue: hi; SP queue: lo terms) so no engine
            # sequencer is blocked by a burst of DMA issues at startup
            def load_w_piece(q):
                cs = slice(q * 2048, (q + 1) * 2048)
                nc.gpsimd.dma_start(w1hi[:, q * 2:(q + 1) * 2, :, :], w1hi_d[:, cs])
                nc.sync.dma_start(w1la[:, q * 2:(q + 1) * 2, :, :, :], w1la_d[:, cs])
                if N_LO == 2:
                    nc.sync.dma_start(w1lb[:, q * 2:(q + 1) * 2, :, :, :], w1lb_d[:, cs])

            load_w_piece(0)
            # s1 state init: s1[p, c, b] = s10[p, c]
            nc.vector.memset(s1[:], 0.0)
            nc.vector.memset(cnt[:], 0.0)
            for c in range(NCH):
                nc.vector.tensor_scalar(s1[:, c, :], s1[:, c, :], s10[:, c:c + 1], None, ADD)

            # ---- wave descriptors (flat across groups) ----
            waves = []
            for gi, (t0, glen) in enumerate(GROUPS):
                cpw = 4 if glen == 2 else 2
                nw = NCH // cpw
                for w in range(nw):
                    waves.append(dict(
                        g=gi, glen=glen, cpw=cpw, c0=w * cpw,
                        first=(w == 0), last=(w == nw - 1)))
            NW = len(waves)

            xt16 = [None] * len(GROUPS)
            xt8 = [None] * len(GROUPS)
            xoff = [0] * len(GROUPS)
            off = 0
            for gi, (t0, glen) in enumerate(GROUPS):
                xoff[gi] = off
                off += 8 * glen * 128

            def load_x(gi):
                glen = GROUPS[gi][1]
                xt = xpool.tile([128, KC, glen * 128], F16, tag="x16", name=f"x16_{gi}")
                x8t = xpool.tile([128, JC, 2, glen * 128], F8E5, tag="x8", name=f"x8_{gi}")
                o = xoff[gi]
                nc.sync.dma_start(xt[:], x16_d[:, o:o + 8 * glen * 128])
                nc.gpsimd.dma_start(x8t[:], x8_d[:, o:o + 8 * glen * 128])
                xt16[gi], xt8[gi] = xt, x8t

            load_x(0)

            # per-wave tile handles, filled during emission
            Pt = [None] * NW                      # l1 psum tile
            SG = [[None] * 3 for _ in range(NW)]  # sigma tiles per tl
            SPK = [[None] * 3 for _ in range(NW)]  # spike tiles per tl
            o2t = {}                              # (gi, tl) -> l2 psum tile

            def emit_l1(i):
                wv = waves[i]
                gi, cpw, c0, glen = wv["g"], wv["cpw"], wv["c0"], wv["glen"]
                if wv["first"] and gi >= 1 and gi + 1 < len(GROUPS):
                    load_x(gi + 1)
                shape = [128, cpw, glen, 128]
                pad = [128, cpw, glen, 128] if glen == 2 else [128, cpw, 4, 128]
                P = l1pool.tile(shape, F32, tag="P", name=f"P_{i}", padded_shape=pad)
                Pt[i] = P
                xt, x8t = xt16[gi], xt8[gi]
                for ci in range(cpw):
                    c = c0 + ci
                    for k in range(KC):
                        nc.tensor.matmul(P[:, ci, :, :], w1hi[:, c, k, :], xt[:, k, :],
                                         start=(k == 0), stop=False)
                    los = (w1la, w1lb) if N_LO == 2 else (w1la,)
                    for lo in los:
                        for j in range(JC):
                            nc.tensor.matmul(P[:, ci, :, :], lo[:, c, j, :, :],
                                             x8t[:, j, :, :], start=False,
                                             stop=(lo is los[-1] and j == JC - 1),
                                             perf_mode=DRMODE)

            # All recurrence ops are per-chunk: each chunk's op1 -> sigma ->
            # spk -> op3 chain completes shortly after that chunk's matmuls,
            # so the psum tile recycles ~2us after the wave instead of riding
            # a 4-chunk grouped barrier through three engines.
            def emit_op1(j, tl):
                # half-wave granularity: short psum-recycle chain at ~25% less
                # DVE issue overhead than per-chunk
                wv = waves[j]
                c0, cpw = wv["c0"], wv["cpw"]
                h = max(1, cpw // 2)
                for ci in range(0, cpw, h):
                    sl = s1[:, c0 + ci:c0 + ci + h, :]
                    nc.vector.tensor_tensor(sl, sl, Pt[j][:, ci:ci + h, tl, :], ADD)

            def emit_op3(j, tl):
                wv = waves[j]
                c0, cpw = wv["c0"], wv["cpw"]
                h = max(1, cpw // 2)
                for ci in range(0, cpw, h):
                    sl = s1[:, c0 + ci:c0 + ci + h, :]
                    nc.vector.scalar_tensor_tensor(sl, sl, BETA,
                                                   SPK[j][tl][:, ci:ci + h, :],
                                                   MULT, SUB)

            def emit_sigma(j, tl):
                wv = waves[j]
                c0, cpw = wv["c0"], wv["cpw"]
                sg = gpool.tile([128, cpw, 128], F16, tag="sg", name=f"sg_{j}_{tl}")
                SG[j][tl] = sg
                for ci in range(cpw):
                    c = c0 + ci
                    nc.scalar.activation(sg[:, ci, :], s1[:, c, :], SIGN,
                                         bias=nth[:, c:c + 1], scale=1.0)

            def emit_spk(j, tl):
                wv = waves[j]
                cpw = wv["cpw"]
                sp = kpool.tile([128, cpw, 128], F16, tag="spk", name=f"spk_{j}_{tl}")
                SPK[j][tl] = sp
                for ci in range(cpw):
                    nc.gpsimd.tensor_scalar(sp[:, ci, :], SG[j][tl][:, ci, :],
                                            0.0, None, MAX)

            def emit_l2(j, tl):
                wv = waves[j]
                gi, c0, cpw = wv["g"], wv["c0"], wv["cpw"]
                if c0 == 0:
                    o2t[(gi, tl)] = o2pool.tile([128, 128], F32, tag="o2",
                                                name=f"o2_{gi}_{tl}")
                o2 = o2t[(gi, tl)]
                sp = SPK[j][tl]
                for ci in range(cpw):
                    jj = c0 + ci
                    nc.tensor.matmul(o2[:], sp[:, ci, :], w2c[:, jj * 128:(jj + 1) * 128],
                                     start=(jj == 0), stop=(jj == L2J - 1))

            def emit_l2rec(gi, tl):
                # returns a list of closures so the caller can spread the DVE
                # work over several slots (keeps DVE under its slot budget)
                o2 = o2t.pop((gi, tl))
                q2 = mpool.tile([128, OUT], F32, tag="q2", name=f"q2_{gi}_{tl}")
                spk2 = mpool.tile([128, OUT], F32, tag="spk2", name=f"spk2_{gi}_{tl}")
                return [
                    lambda: nc.vector.scalar_tensor_tensor(q2[:], o2[:, OUT:], 1.0 / S2, s2[:], MULT, ADD),
                    lambda: nc.vector.tensor_tensor(q2[:], q2[:], o2[:, :OUT], ADD),
                    lambda: nc.vector.tensor_tensor(spk2[:], q2[:], th2[:], ISGT),
                    lambda: nc.vector.scalar_tensor_tensor(s2[:], q2[:], BETA, spk2[:], MULT, SUB),
                    lambda: nc.vector.tensor_tensor(cnt[:], cnt[:], spk2[:], ADD),
                ]

            def has_tl(j, d):
                return 0 <= j < NW and d < waves[j]["glen"]

            # ---- software-pipelined emission ----
            l2_emitted = set()
            l2rec_ops = []
            for i in range(NW + 8):
                # deferred layer-2 recurrence ops, at most 3 per slot so the
                # DVE ladder never exceeds its slot budget
                budget = 3 if i < NW - 2 else 16
                while l2rec_ops and budget > 0:
                    l2rec_ops.pop(0)()
                    budget -= 1
                if 0 <= i <= 14:
                    load_w_piece(i + 1)
                if i == 0:
                    nc.sync.dma_start(w2c[:], w2c_d[:, :])
                if i == 8:
                    load_x(1)
                if i < NW:
                    emit_l1(i)
                # DVE ladder: op3 one slot after spike, op1 chained after op3
                for d in range(1, 4):
                    j = i - d
                    if has_tl(j, d - 1):
                        emit_op3(j, d - 1)
                    if has_tl(j, d):
                        emit_op1(j, d)
                if i < NW:
                    emit_op1(i, 0)
                # ACT sigmas for the op1s emitted this slot (t0 last)
                for d in range(1, 4):
                    if has_tl(i - d, d):
                        emit_sigma(i - d, d)
                if i < NW:
                    emit_sigma(i, 0)
                # Pool spikes in same order
                for d in range(1, 4):
                    if has_tl(i - d, d):
                        emit_spk(i - d, d)
                if i < NW:
                    emit_spk(i, 0)
                # layer-2 matmuls: lag 2 slots behind spike production (lag 1
                # for the final waves so the tail drains sooner); then the
                # layer-2 recurrence immediately after a group's last chunk
                for j in range(max(0, i - 6), i + 1):
                    if not (0 <= j < NW):
                        continue
                    lag = 1 if j >= NW - 4 else 2
                    for tl in range(waves[j]["glen"]):
                        if (j, tl) in l2_emitted or i < j + tl + lag:
                            continue
                        l2_emitted.add((j, tl))
                        emit_l2(j, tl)
                        if waves[j]["last"]:
                            l2rec_ops.extend(emit_l2rec(waves[j]["g"], tl))

            nc.sync.dma_start(out_d[:, :], cnt[:])

    nc.compile()
    return nc


def _prep_shared(w1, b1, w2, b2):
    w1 = w1.astype(np.float32)
    hi16 = w1.astype(np.float16)
    hiF = hi16.astype(np.float32)
    resA = ((w1 - hiF) * np.float32(S1)).astype(E4NP)
    resB = ((w1 - hiF - resA.astype(np.float32) / np.float32(S1)) * np.float32(S1)).astype(E4NP)

    # w1hi: [p, c*1024 + k*128 + n] = hi16[c*128+n, k*128+p]
    def lay_hi(a):
        return np.ascontiguousarray(
            a.reshape(NCH, 128, KC, 128).transpose(3, 0, 2, 1).reshape(128, NCH * 1024))

    # lo: [p, c*1024 + j*256 + s*128 + n] = lo[c*128+n, j*256+s*128+p]
    def lay_lo(a):
        return np.ascontiguousarray(
            a.reshape(NCH, 128, JC, 2, 128).transpose(4, 0, 2, 3, 1).reshape(128, NCH * 1024))

    w2t = np.ascontiguousarray(w2.T.astype(np.float32))       # [HID, OUT]
    w2hi = w2t.astype(np.float16)
    w2lo = ((w2t - w2hi.astype(np.float32)) * np.float32(S2)).astype(np.float16)
    w2cat = np.concatenate([w2hi.reshape(L2J, 128, OUT), w2lo.reshape(L2J, 128, OUT)],
                           axis=2)
    w2c = np.ascontiguousarray(w2cat.transpose(1, 0, 2).reshape(128, L2J * 128))

    c1 = (b1.astype(np.float32) / np.float32(1.0 - BETA)).astype(np.float32)
    c2 = (b2.astype(np.float32) / np.float32(1.0 - BETA)).astype(np.float32)
    nth = np.ascontiguousarray((c1 - np.float32(1.0)).reshape(NCH, 128).T)  # -(1-c1)
    s10 = np.ascontiguousarray((-np.float32(BETA) * c1).reshape(NCH, 128).T)
    th2 = np.broadcast_to((1.0 - c2).astype(np.float32), (128, OUT))
    s20 = np.broadcast_to((-BETA * c2).astype(np.float32), (128, OUT))
    sml = np.ascontiguousarray(
        np.concatenate([nth, s10, th2, s20], axis=1).astype(np.float32))
    out = {"w1hi": lay_hi(hi16), "w1la": lay_lo(resA), "w2c": w2c, "sml": sml}
    if N_LO == 2:
        out["w1lb"] = lay_lo(resB)
    return out


def _prep_x(spike_seq, core):
    xs = np.asarray(spike_seq[:, core * BC:(core + 1) * BC, :], dtype=np.float32)
    x16_blocks, x8_blocks = [], []
    for t0, glen in GROUPS:
        blk = xs[t0:t0 + glen]                       # [glen, 128b, 1024in]
        b16 = blk.reshape(glen, 128, KC, 128).transpose(3, 2, 0, 1)   # [p, k, tl, b]
        x16_blocks.append(b16.reshape(128, KC * glen * 128))
        b8 = blk.reshape(glen, 128, JC, 2, 128).transpose(4, 2, 3, 0, 1)  # [p,j,s,tl,b]
        x8_blocks.append((b8 * X8V).reshape(128, JC * 2 * glen * 128))
    x16 = np.ascontiguousarray(np.concatenate(x16_blocks, axis=1)).astype(np.float16)
    x8 = np.ascontiguousarray(np.concatenate(x8_blocks, axis=1)).astype(E5NP)
    return x16, x8


def kernel(spike_seq, w1, b1, w2, b2):
    if "nc" not in _CACHE:
        _CACHE["nc"] = _build_nc()
    nc = _CACHE["nc"]

    shared = _prep_shared(np.asarray(w1), np.asarray(b1), np.asarray(w2), np.asarray(b2))
    in_maps = []
    for c in range(NCORES):
        x16, x8 = _prep_x(spike_seq, c)
        in_maps.append({"x16": x16, "x8": x8, **shared})
    res = run_bass_kernel_spmd(nc, in_maps, core_ids=list(range(NCORES)))
    out = np.concatenate([res.results[c]["out"] for c in range(NCORES)], axis=0)
    return out.astype(np.asarray(spike_seq).dtype)
